# revision 40
# baseline (speedup 1.0000x reference)
"""
DistancePredictor Trainium2 kernel.

Math:
  xi = x @ Wi + bi            [B, L, H]
  xj = x @ Wj + bj            [B, L, H]
  out = relu(xi[:,:,None,:] * xj[:,None,:,:]) @ Wo + bo    [B, L, L, NB]

Key identity (exact, terms have disjoint support):
  relu(a*b) = relu(a)relu(b) + relu(-a)relu(-b)
so
  out[i,j,n] = sum_h (A+[i,h]B+[j,h] + A-[i,h]B-[j,h]) * Wo[h,n] + bo[n]
with A± = relu(±xi), B± = relu(±xj) — the whole pair/relu/contract
pipeline is pure TensorE matmuls; no [B,L,L,H] intermediate exists.

Sharding: 8 cores; core c handles batch b=c//4 and i-rows
[96*(c%4), 96*(c%4)+96).  Weights replicated.

Schedule (v2 — k-major unified stream):
 - All inputs stream as 10 chunk triggers chk[k] = [wi_k | wj_k | x_k]
   (229KB each, 1792B/partition descriptors), alternating the two HWDGE
   rings.  Chunk k's arrival enables ALL of layer-1 for contraction
   chunk k (psA t0/t1 + psB t0/t1, single-pass LDWEIGHTS), so layer 1
   is stream-paced and finishes ~0.5us after the last byte.
 - cst/bias rows ride the gpsimd SWDGE path (off the HW rings).
 - Biases enter the PSUM accumulations as rank-1 matmuls (ones ⊗ b).
 - Junk matmuls on a memset tile ramp the HAM clock before chunk 0.
 - at±-chain split: Vector owns t0 (fused PSUM reads), GpSimd owns t1
   (from SBUF relu copies), Scalar owns am1/bp relus.
 - Main contraction is j-block major: stationary = b±t j-block (M=128),
   moving = at± [n-half, i] (N=480).
 - bo + fp32->fp16 conversion fuse into one Vector broadcast-add per
   output block; blocks drain on alternating rings during the main loop.
"""

import numpy as np

import concourse.bass as bass
import concourse.mybir as mybir
import concourse.tile as tile
from concourse import bacc, bass_utils

# Problem constants (hardcoded per contract).
B, L, D, H, NB = 2, 384, 1280, 256, 10
P = 128
KT = D // P     # 10 contraction chunks of 128
HT = H // P     # 2 h-chunks of 128
NCORES = 8
IB = (B * L) // NCORES   # 96 i-rows per core
CW = 2 * H + L           # chunk width: wi(256) | wj(256) | x(384)

F32 = mybir.dt.float32
F16 = mybir.dt.float16
ALU = mybir.AluOpType
RELU = mybir.ActivationFunctionType.Relu

_last_result = None  # BassKernelResults of the most recent run (for test harness)


def build_nc():
    nc = bacc.Bacc("TRN2")

    # A-chunks (per k: [wi_k(0:256) | xi_k(256:352)]): two quad-triggers
    # (2816B descriptors, matching chb's 2560B for ring fairness) + one
    # pair-trigger for k8,9
    cha_q = nc.dram_tensor("cha_q", [2, P, 4, 352], F16, kind="ExternalInput")
    cha_p = nc.dram_tensor("cha_p", [P, 2, 352], F16, kind="ExternalInput")
    # B-chunk pair g: per partition [wj_k(0:256) | x_k(256:640)]
    chb = nc.dram_tensor("chb", [KT // 2, P, 2, 640], F16, kind="ExternalInput")
    # cst[:, 0:2] = Wo per h-chunk, [:, 2:4] = -Wo, [:, 4] = bo replicated
    cst = nc.dram_tensor("cst", [P, 5, NB], F32, kind="ExternalInput")
    # bias rows on one partition: [bi_t0, bi_t1, bj_t0, bj_t1]
    brow = nc.dram_tensor("brow", [1, 4, P], F16, kind="ExternalInput")
    # [n-half, j-block, j, n, i]: output in j-major blocks (M=128 matmuls)
    out = nc.dram_tensor("out", [2, 3, P, NB // 2, IB], F16, kind="ExternalOutput")

    with tile.TileContext(nc) as tc:
        with (
            tc.tile_pool(name="persist", bufs=1) as pp,
            tc.tile_pool(name="psA", bufs=2, space="PSUM") as psA_pool,
            tc.tile_pool(name="psB", bufs=2, space="PSUM") as psB_pool,
            tc.tile_pool(name="psO", bufs=4, space="PSUM") as psO_pool,
            tc.tile_pool(name="stage", bufs=6) as stage_pool,
        ):
            tl = lambda shape, name, dt=F32: pp.tile(shape, dt, name=name, tag=name)
            cha_sb = tl([P, KT // 2, 2, 352], "cha_sb", F16)
            chb_sb = tl([P, KT // 2, 2, 640], "chb_sb", F16)
            cst_sb = tl([P, 5, NB], "cst_sb")
            brow_sb = tl([1, 4, P], "brow_sb", F16)
            ones_sb = tl([1, L], "ones_sb", F16)

            bp_sb = tl([P, HT, L], "bp_sb", F16)         # relu(xj+bj)      [h, j]
            bm_sb = tl([P, HT, L], "bm_sb", F16)         # relu(-(xj+bj))
            atp_sb = tl([P, HT, NB, IB], "atp_sb", F16)  # [h, n, i]
            atm_sb = tl([P, HT, NB, IB], "atm_sb", F16)
            ap1_sb = tl([P, IB], "ap1_sb")               # max(psA1, 0)
            am1_sb = tl([P, IB], "am1_sb")               # relu(-psA1)

            warm_sb = tl([P, L], "warm_sb", F16)
            # memsets on gpsimd: it is idle at program start, so the first
            # junk matmul (and the HAM clock ramp) starts ~1.5us earlier
            # than with vector memsets (vector waits on engine-main entry).
            # Only the stationary 96 columns need zeroing (all-zero
            # stationary makes the junk product zero regardless of the
            # uninitialized moving columns); the short memset unblocks
            # the first junk matmul ~0.3us earlier.
            nc.gpsimd.memset(warm_sb[:, :IB], 0.0)
            nc.gpsimd.memset(ones_sb[:], 1.0)

            def junk(n_junk):
                # Full 128-partition matmuls: the HAM clock monitor only
                # counts wide-K PE streaming (K=32/64 never ramps).
                psW = psO_pool.tile([IB, L], F32, name="psW", tag="psO")
                for _ in range(n_junk):
                    nc.tensor.matmul(psW[:], warm_sb[:, :IB], warm_sb[:],
                                     start=True, stop=True,
                                     skip_group_check=True)

            # ---- DMA triggers.  Emission order per engine = issue order.
            # Two-phase stream: A-chunks (wi + i-cols of x) first so psA
            # closes ~3us in and the long at±-chain overlaps the B-phase;
            # B-chunks (wj + full x) stream-pace psB, which closes right
            # before the main loop.  brow lands early on the scalar ring
            # (bias matmuls run mid-A-phase); cst mid-A on sync (needed
            # by the at±-chain at ~A-end).
            # Ring sharing is packet-fair (not byte-fair): both rings must
            # carry the SAME kind of data at each moment or the one with
            # bigger descriptors hogs bandwidth.  Alternate groups within
            # each phase; a single ring is descgen-limited (~250GB/s).
            # Interleaved stream on desc-size-matched rings (A quads
            # 2816B / B halves 2560B); cst+brow ride the off-ring SWDGE
            # path (latency ~2us, needed only mid-stream).  B j-halves
            # split across both rings so B-bytes pace the whole window.
            nc.gpsimd.dma_start(brow_sb[:], brow[:])
            nc.gpsimd.dma_start(cst_sb[:], cst[:])
            nc.sync.dma_start(cha_sb[:, 0:2], cha_q[0])
            nc.scalar.dma_start(cha_sb[:, 2:4], cha_q[1])
            nc.sync.dma_start(cha_sb[:, 4], cha_p[:])
            nc.scalar.dma_start(chb_sb[:, 0, 0], chb[0, :, 0])
            nc.sync.dma_start(chb_sb[:, 0, 1], chb[0, :, 1])
            nc.scalar.dma_start(chb_sb[:, 1, 0], chb[1, :, 0])
            nc.sync.dma_start(chb_sb[:, 1, 1], chb[1, :, 1])
            for g in range(2, KT // 2):
                nc.scalar.dma_start(chb_sb[:, g, 0], chb[g, :, 0])
                nc.sync.dma_start(chb_sb[:, g, 1], chb[g, :, 1])

            psA = [psA_pool.tile([P, IB], F32, name="psA", tag="psA")
                   for _ in range(HT)]
            psB = [psB_pool.tile([P, L], F32, name="psB", tag="psB")
                   for _ in range(HT)]

            # ~3.5us of dense junk: the HAM un-throttle needs one fully-busy
            # 3.4us window of wide-N PE streaming, and the N=96 A-phase
            # matmuls never provide it.  This burns the dead stream-head
            # time and guarantees the A-phase runs at 2.4GHz.
            junk(9)

            # ---- layer 1 A-side: psA chunk-paced on the A-stream; bias
            # rank-1 joins after group 0; psA[0] closes first so the
            # Vector t0-chain starts while the t1 matmuls finish.
            def a_mm(g, j, t, start=False, stop=False):
                nc.tensor.matmul(psA[t][:], cha_sb[:, g, j, t * P:(t + 1) * P],
                                 cha_sb[:, g, j, H:H + IB],
                                 start=start, stop=stop)

            def b_mm(g, j, t, start=False, stop=False):
                nc.tensor.matmul(psB[t][:],
                                 chb_sb[:, g, j, t * P:(t + 1) * P],
                                 chb_sb[:, g, j, H:],
                                 start=start, stop=stop)

            for g in range(KT // 2):
                last = g == KT // 2 - 1
                if not last:
                    for j in range(2):
                        for t in range(HT):
                            a_mm(g, j, t, start=g == 0 and j == 0)
                else:
                    # close t0 before t1 so the t0 post-ops start first
                    a_mm(g, 0, 0)
                    a_mm(g, 0, 1)
                    a_mm(g, 1, 0, stop=True)
                    a_mm(g, 1, 1, stop=True)
                if g == 1:
                    for t in range(HT):
                        nc.tensor.matmul(psA[t][:], brow_sb[:, t],
                                         ones_sb[:, :IB],
                                         start=False, stop=False)
                junk(1)

            # ---- at±-chain: emitted here (= high scheduler priority) so
            # it runs concurrently with the B-phase stream/matmuls.
            wo_b = lambda s, lo, hi: cst_sb[:, s, lo:hi, None].to_broadcast(
                (P, hi - lo, IB))
            psa_b = lambda lo, hi: psA[0][:, None, :].to_broadcast(
                (P, hi - lo, IB))

            def at0_op(sign, lo, hi):
                # fused t0: atp = max(psA,0)*Wo ; atm = min(psA,0)*(-Wo)
                dst = (atp_sb if sign == 0 else atm_sb)[:, 0, lo:hi]
                op0 = ALU.max if sign == 0 else ALU.min
                nc.vector.scalar_tensor_tensor(dst, psa_b(lo, hi), 0.0,
                                               wo_b(2 * sign, lo, hi),
                                               op0, ALU.mult)

            def at1_op(sign, lo, hi, eng=None):
                # t1 from SBUF relu copies; both a-parts non-negative -> +Wo.
                src = ap1_sb if sign == 0 else am1_sb
                dst = (atp_sb if sign == 0 else atm_sb)[:, 1, lo:hi]
                (eng or nc.gpsimd).tensor_tensor(
                    dst, src[:, None, :].to_broadcast((P, hi - lo, IB)),
                    wo_b(1, lo, hi), ALU.mult)

            # no scalar.activation anywhere: the ACT table load (1.3us on
            # the Scalar engine) delays the scalar HWDGE ring's doorbell
            # by ~2us, starving the stream's second ring
            nc.vector.tensor_scalar(am1_sb[:], psA[1][:], -1.0, 0.0,
                                    ALU.mult, ALU.max)
            at0_op(0, 0, 5)
            nc.vector.tensor_scalar_max(ap1_sb[:], psA[1][:], 0.0)
            at0_op(1, 0, 5)
            at0_op(0, 5, 10)
            at0_op(1, 5, 10)
            at1_op(0, 0, 5)
            at1_op(1, 0, 5)
            at1_op(0, 5, 10)
            # vector: gpsimd's 4-op serial chain would deliver this after
            # the main loop already needs it
            at1_op(1, 5, 10, eng=nc.vector)

            # ---- layer 1 B-side: psB chunk-paced on the B-stream ----
            for g in range(KT // 2):
                last = g == KT // 2 - 1
                if not last:
                    for j in range(2):
                        for t in range(HT):
                            b_mm(g, j, t, start=g == 0 and j == 0)
                else:
                    b_mm(g, 0, 0)
                    b_mm(g, 0, 1)
                    b_mm(g, 1, 0, stop=True)
                    b_mm(g, 1, 1, stop=True)
                if g == 0:
                    for t in range(HT):
                        nc.tensor.matmul(psB[t][:], brow_sb[:, 2 + t],
                                         ones_sb[:],
                                         start=False, stop=False)


            # ---- b± relus: scalar owns bp, vector owns bm; split per
            # j-block so the main loop's first matmuls are gated by a
            # ~200ns slice relu instead of a full-L one.
            # all 12 on scalar: vector's queue (at-chain + psO drains)
            # must stay clear; slice order matches main-loop consumption
            for jb in range(3):
                js = slice(jb * P, (jb + 1) * P)
                nc.vector.tensor_scalar_max(bp_sb[:, 0, js], psB[0][:, js],
                                             0.0)
                nc.vector.tensor_scalar(bm_sb[:, 0, js], psB[0][:, js],
                                        -1.0, 0.0, ALU.mult, ALU.max)
                nc.vector.tensor_scalar_max(bp_sb[:, 1, js], psB[1][:, js],
                                             0.0)
                nc.vector.tensor_scalar(bm_sb[:, 1, js], psB[1][:, js],
                                        -1.0, 0.0, ALU.mult, ALU.max)

            # ---- main contraction, j-block major: stationary = b±t j-block
            # (M=128), moving = at± [n-half, i] (N=480).  Output bias
            # enters via cst row 4 during the fp32->fp16 PSUM drain.
            NH2 = NB // 2
            atp_v = atp_sb[:].rearrange("p t n i -> p t (n i)")
            atm_v = atm_sb[:].rearrange("p t n i -> p t (n i)")
            junk(1)
            for idx in range(6):
                nh, jb = idx // 3, idx % 3
                ns = slice(nh * NH2 * IB, (nh + 1) * NH2 * IB)
                js = slice(jb * P, (jb + 1) * P)
                psO = psO_pool.tile([P, NH2 * IB], F32, name="psO", tag="psO")
                nc.tensor.matmul(psO[:], bp_sb[:, 0, js], atp_v[:, 0, ns],
                                 start=True, stop=False)
                if idx == 0:
                    junk(2)  # bridge the at-chain latency, keep the clock up
                nc.tensor.matmul(psO[:], bm_sb[:, 0, js], atm_v[:, 0, ns],
                                 start=False, stop=False)
                if idx == 0:
                    junk(1)
                nc.tensor.matmul(psO[:], bp_sb[:, 1, js], atp_v[:, 1, ns],
                                 start=False, stop=False)
                if idx == 0:
                    junk(1)
                nc.tensor.matmul(psO[:], bm_sb[:, 1, js], atm_v[:, 1, ns],
                                 start=False, stop=True)
                # bias rides the fp16 convert as a broadcast add on Vector
                # (cst row 4 = bo replicated across partitions); no PE time.
                ostage = stage_pool.tile([P, NH2, IB], F16, name="ostage",
                                         tag="ostage")
                psO_3 = psO[:].rearrange("p (n i) -> p n i", i=IB)
                bo_b = lambda lo, hi: cst_sb[:, 4, nh * NH2 + lo:nh * NH2 + hi,
                                             None].to_broadcast((P, hi - lo, IB))
                eng = nc.scalar if idx % 2 == 0 else nc.sync
                if idx >= 4:
                    # last two blocks: convert/DMA in slices alternating
                    # rings so the tail pipeline drains concurrently
                    engs = [nc.scalar, nc.sync]
                    cuts = [0, 2, 4, NH2]
                    for s in range(3):
                        lo, hi = cuts[s], cuts[s + 1]
                        nc.vector.tensor_tensor(ostage[:, lo:hi],
                                                psO_3[:, lo:hi],
                                                bo_b(lo, hi), ALU.add)
                        engs[(idx + s) % 2].dma_start(out[nh, jb, :, lo:hi],
                                                      ostage[:, lo:hi])
                else:
                    nc.vector.tensor_tensor(ostage[:], psO_3, bo_b(0, NH2),
                                            ALU.add)
                    eng.dma_start(out[nh, jb], ostage[:])

    return nc


def _prep_inputs(x, Wi, bi, Wj, bj, Wo, bo):
    """Build the 8 per-core input maps."""
    f = lambda a: np.ascontiguousarray(np.asarray(a, dtype=np.float32))
    x, Wi, bi, Wj, bj, Wo, bo = map(f, (x, Wi, bi, Wj, bj, Wo, bo))

    wi_k = Wi.astype(np.float16).reshape(KT, P, H)      # [k, 128, 256]
    wj_k = Wj.astype(np.float16).reshape(KT, P, H)

    wo_r = Wo.reshape(HT, P, NB).transpose(1, 0, 2)            # [128, 2, 10]
    cst = np.ascontiguousarray(np.stack(
        [wo_r[:, 0], wo_r[:, 1], -wo_r[:, 0], -wo_r[:, 1],
         np.tile(bo[None, :], (P, 1))], axis=1)).astype(np.float32)  # [128, 5, 10]
    brow = np.concatenate([bi.reshape(HT, P), bj.reshape(HT, P)],
                          axis=0)[None].astype(np.float16)     # [1, 4, 128]
    brow = np.ascontiguousarray(brow)

    xT = [x[b].T for b in range(B)]                            # [1280, 384]
    in_maps = []
    for c in range(NCORES):
        b, i0 = c // (NCORES // B), (c % (NCORES // B)) * IB
        xc = np.roll(xT[b], -i0, axis=1).astype(np.float16)    # i-cols first
        xk = xc.reshape(KT, P, L)                              # [k, 128, 384]
        # A-chunks: [wi_k | xi_k] as 2 quads + 1 pair; B-pairs: [wj_k | x_k]
        cha = np.concatenate([wi_k, xk[:, :, :IB]], axis=2)    # [k, 128, 352]
        cha_q = np.ascontiguousarray(
            cha[:8].reshape(2, 4, P, 352).transpose(0, 2, 1, 3))
        cha_p = np.ascontiguousarray(cha[8:].transpose(1, 0, 2))
        chb = np.concatenate([wj_k, xk], axis=2)               # [k, 128, 640]
        chb = np.ascontiguousarray(
            chb.reshape(KT // 2, 2, P, 640).transpose(0, 2, 1, 3))
        in_maps.append({"cha_q": cha_q, "cha_p": cha_p, "chb": chb,
                        "cst": cst, "brow": brow})
    return in_maps


def _run(inputs, trace=False):
    global _last_result
    nc = build_nc()
    if not nc.is_finalized():
        nc.finalize()
    in_maps = _prep_inputs(**inputs)
    res = bass_utils.run_bass_kernel_spmd(
        nc, in_maps, core_ids=list(range(NCORES)), trace=trace)
    _last_result = res
    full = np.empty((B, L, L, NB), dtype=np.float32)
    for c in range(NCORES):
        b, i0 = c // (NCORES // B), (c % (NCORES // B)) * IB
        o = res.results[c]["out"].astype(np.float32)   # [2, 3, 128, 5, 96]
        o = o.transpose(4, 1, 2, 0, 3).reshape(IB, L, NB)  # -> [i, j_rolled, n]
        full[b, i0:i0 + IB] = np.roll(o, i0, axis=1)
    return full


def kernel(**inputs):
    return _run(inputs, trace=False)
